# revision 15
# baseline (speedup 1.0000x reference)
"""Chamfer loss on 8 Trainium2 NeuronCores (v3).

pred [8192,3], label [8192,3] fp32 ->
scalar = mean_i min_j ||p_i - l_j|| + mean_j min_i ||p_i - l_j||

Core k owns pred rows [k*1024:(k+1)*1024] and computes one [1024 x 8192]
NEGATED squared-distance block via an augmented K=5 fp32r matmul with
host-precomputed operands:
  U[5,1024] = [ 2x | -1 | -||x||^2 ]   (stationary, 128-row tiles)
  V[5,8192] = [ y  | ||y||^2 | 1 ]     (moving)
so (U^T V)[i,j] = -||x_i - y_j||^2 in fp32 PSUM. Negation makes every
reduction a MAX, which the Pool engine's cross-lane (partition-axis)
TensorReduce supports.

Column-group loop (outer, widths [2048,2048,2048,1536,512]) x row-tile
loop (inner, 8 tiles). Per cell [128,W]:
  - 512-wide matmuls into a [128,2048] PSUM tile (bufs=2)
  - drain to SBUF f16: ACT copy + DVE 4x tensor_scalar scan (clamp min0 +
    row-max accum), or a DVE tensor_scalar drain straight from PSUM
    (fused drain+clamp+scan, 1x) on a few cells to balance ACT vs DVE
  - column fold into the group accumulator: DVE tensor_tensor MAX (2x)
Group tail: Pool TensorReduce over AxisListType.C -> [1,W] per-label
partials, DMA'd per group; the last 512-wide group keeps the terminal
chain short. Pred side: per-(group,tile) row-max slots are folded and
exported raw [128,8]; the host does sqrt+mean (hardware sqrt would cost
an ACT table load and a serial tail).

Host combines: rm -> sqrt+sum per core; label partials pmin(-max) across
cores -> sqrt+mean. Engine assignment tables below are balanced against
the TimelineSim cost model (ACT 0.83ns/elem drain; DVE 0.26 scan /
0.52 fold; Pool 1.39 C-reduce; PE warmup matmuls cover the p-state ramp).
"""

import sys

if "/opt/trn_rl_repo" not in sys.path:
    sys.path.insert(0, "/opt/trn_rl_repo")

import numpy as np

import concourse.bacc as bacc
import concourse.mybir as mybir
from concourse import tile
from concourse.bass_utils import run_bass_kernel_spmd

F32 = mybir.dt.float32
F32R = mybir.dt.float32r
F16 = mybir.dt.float16
MIN = mybir.AluOpType.min
MAX = mybir.AluOpType.max
ADD = mybir.AluOpType.add
AF = mybir.ActivationFunctionType
AX_X = mybir.AxisListType.X
AX_C = mybir.AxisListType.C

N_CORES = 8
N_PTS = 8192
ROWS = N_PTS // N_CORES          # 1024 pred rows per core
N_RT = ROWS // 128               # 8 row tiles

# column groups: (start, width)
GROUPS = [(0, 2048), (2048, 2048), (4096, 2048), (6144, 1536), (7680, 512)]

# drain engine per (group, row tile): 'A' = ACT copy + DVE scan,
# 'V' = DVE tensor_scalar fused drain+scan from PSUM (1x). V-cells sit at
# odd row tiles so their pair-fold naturally follows the V-drain in the
# DVE queue.
DRAIN = [
    "AAAVAVAA",   # g0
    "AAAVAVAA",   # g1
    "AAAVAAAA",   # g2
    "AAAAAVAA",   # g3 (1536)
    "AAAAAAAA",   # g4 (512, terminal)
]
# fold style per group: 'pair' = 4 independent pair-folds + per-pair Pool
# C-reduce into a [4,w] strip + strip C-reduce (shifts fold work to Pool);
# 'chain' = sequential 7-fold chain + single Pool C-reduce.
FOLDS = ["pair", "pair", "pair", "chain", "chain"]


def build_program():
    nc = bacc.Bacc(
        "TRN2",
        target_bir_lowering=False,
        debug=False,
        enable_asserts=False,
        num_devices=N_CORES,
    )
    u_d = nc.dram_tensor("u_d", (5, ROWS), F32R, kind="ExternalInput")
    v_d = nc.dram_tensor("v_d", (5, N_PTS), F32R, kind="ExternalInput")
    rm_d = nc.dram_tensor("rm_d", (128, N_RT), F32, kind="ExternalOutput")
    # flat label partials: 3 pair-groups x 4 pairs x 2048, then 1536 + 512
    lm = nc.dram_tensor("lm", (1, 26624), F16, kind="ExternalOutput")

    with tile.TileContext(nc) as tc:
        with tc.tile_pool(name="const", bufs=1) as const_pool:
            # operand staging: first matmul needs U + V[:, :512] -> load those
            # first on the SP queue; the rest rides the Pool DGE queue.
            U = const_pool.tile([5, ROWS], F32R)
            nc.scalar.dma_start(U[:], u_d.ap())
            V0 = const_pool.tile([5, 2048], F32R, tag="v0", name="v_0")
            nc.sync.dma_start(V0[:], v_d.ap()[:, 0:2048])
            Vs = [V0]
            for g, (c0, w) in enumerate(GROUPS):
                if g == 0:
                    continue
                V = const_pool.tile([5, w], F32R, tag=f"v{g}", name=f"v_{g}")
                nc.gpsimd.dma_start(V[:], v_d.ap()[:, c0 : c0 + w])
                Vs.append(V)

            with (
                tc.tile_pool(name="mm", bufs=2, space="PSUM") as mm_pool,
                tc.tile_pool(name="s", bufs=3) as s_pool,
                tc.tile_pool(name="acc", bufs=2) as acc_pool,
                tc.tile_pool(name="small", bufs=2) as small_pool,
                tc.tile_pool(name="misc", bufs=1) as misc_pool,
            ):
                trash = misc_pool.tile([128, 2048], F16)
                lmv = misc_pool.tile([1, 26624], F16)
                slots = misc_pool.tile([128, N_RT * len(GROUPS)], F32)

                for g, (c0, w) in enumerate(GROUPS):
                    pair = FOLDS[g] == "pair"
                    acc = None          # chain accumulator
                    dsts = {}           # r -> drained tile
                    pacc = {}           # pair index -> pair accumulator
                    for r in range(N_RT):
                        mm = mm_pool.tile([128, 2048], F32, tag="mm",
                                          name=f"mm_{g}_{r}")
                        for j in range(w // 512):
                            nc.tensor.matmul(
                                mm[:, j * 512 : (j + 1) * 512],
                                U[:, r * 128 : (r + 1) * 128],
                                Vs[g][:, j * 512 : (j + 1) * 512],
                                start=True,
                                stop=True,
                            )
                        if not pair and r == 0:
                            dst = acc_pool.tile([128, w], F16, tag=f"acc{g}",
                                                name=f"acc_{g}_{r}")
                        else:
                            dst = s_pool.tile([128, w], F16, tag=f"s{g}",
                                              name=f"s_{g}_{r}")
                        slot = slots[:, g * N_RT + r : g * N_RT + r + 1]
                        if DRAIN[g][r] == "A":
                            nc.scalar.copy(dst[:], mm[:, :w])
                            nc.vector.tensor_scalar(
                                out=trash[:, :w], in0=dst[:],
                                scalar1=0.0, scalar2=None,
                                op0=MIN, op1=MAX, accum_out=slot,
                            )
                        else:
                            nc.vector.tensor_scalar(
                                out=dst[:], in0=mm[:, :w],
                                scalar1=0.0, scalar2=None,
                                op0=MIN, op1=MAX, accum_out=slot,
                            )
                        dsts[r] = dst
                        if pair:
                            if r % 2 == 1:
                                pi = r // 2
                                pt = acc_pool.tile([128, w], F16,
                                                   tag=f"p{pi}",
                                                   name=f"pacc_{g}_{pi}")
                                nc.vector.tensor_tensor(
                                    out=pt[:], in0=dsts[r - 1][:],
                                    in1=dst[:], op=MAX,
                                )
                                pacc[pi] = pt
                        else:
                            if r == 0:
                                acc = dst
                            else:
                                nacc = acc_pool.tile([128, w], F16,
                                                     tag=f"acc{g}",
                                                     name=f"acc_{g}_{r}")
                                nc.vector.tensor_tensor(
                                    out=nacc[:], in0=acc[:], in1=dst[:],
                                    op=MAX,
                                )
                                acc = nacc

                    if pair:
                        # per-pair Pool C-reduce into free-offset segments of
                        # the flat partial vector; host maxes over pairs
                        base = g * 4 * 2048
                        for pi in range(4):
                            nc.gpsimd.tensor_reduce(
                                lmv[0:1, base + pi * w : base + (pi + 1) * w],
                                pacc[pi][:], axis=AX_C, op=MAX,
                            )
                        nc.sync.dma_start(
                            lm.ap()[0:1, base : base + 4 * w],
                            lmv[0:1, base : base + 4 * w],
                        )
                    elif g < len(GROUPS) - 1:
                        nc.gpsimd.tensor_reduce(
                            lmv[0:1, 24576 : 24576 + w], acc[:], axis=AX_C,
                            op=MAX,
                        )
                        nc.sync.dma_start(
                            lm.ap()[0:1, 24576 : 24576 + w],
                            lmv[0:1, 24576 : 24576 + w],
                        )
                    else:
                        # terminal group: chunk the tail so the last piece
                        # (reduce + DMA) is small
                        for h0, hw in ((0, 384), (384, 128)):
                            cs = 26112 + h0
                            nc.gpsimd.tensor_reduce(
                                lmv[0:1, cs : cs + hw],
                                acc[:, h0 : h0 + hw],
                                axis=AX_C, op=MAX,
                            )
                            nc.sync.dma_start(
                                lm.ap()[0:1, cs : cs + hw],
                                lmv[0:1, cs : cs + hw],
                            )

                # pred tail: fold per-group slots -> [128, 8] row maxes of -d^2
                ns = len(GROUPS)
                rm = small_pool.tile([128, N_RT], F32, tag="rm")
                nc.vector.tensor_tensor(
                    out=rm[:],
                    in0=slots[:, 0:N_RT],
                    in1=slots[:, N_RT : 2 * N_RT],
                    op=MAX,
                )
                for g in range(2, ns):
                    nrm = small_pool.tile([128, N_RT], F32, tag="rm",
                                          name=f"rm_{g}")
                    nc.vector.tensor_tensor(
                        out=nrm[:], in0=rm[:],
                        in1=slots[:, g * N_RT : (g + 1) * N_RT], op=MAX,
                    )
                    rm = nrm
                nc.scalar.dma_start(rm_d.ap(), rm[:])

    nc.compile()
    return nc


_NC_CACHE = None


def _run(pred: np.ndarray, label: np.ndarray, trace: bool = False):
    global _NC_CACHE
    if _NC_CACHE is None:
        _NC_CACHE = build_program()
    nc = _NC_CACHE

    pred = np.ascontiguousarray(pred, dtype=np.float32)
    label = np.ascontiguousarray(label, dtype=np.float32)

    # host-side augmented operands (negated pred side -> -d^2 on device)
    y2 = (label * label).sum(axis=1)
    V = np.empty((5, N_PTS), np.float32)
    V[0:3] = label.T
    V[3] = y2
    V[4] = 1.0

    in_maps = []
    for k in range(N_CORES):
        x = pred[k * ROWS : (k + 1) * ROWS]
        x2 = (x * x).sum(axis=1)
        Uk = np.empty((5, ROWS), np.float32)
        Uk[0:3] = 2.0 * x.T
        Uk[3] = -1.0
        Uk[4] = -x2
        in_maps.append({"u_d": Uk, "v_d": V})

    # transient NRT_EXEC_UNIT_UNRECOVERABLE on first touch after idling;
    # retry on a fresh dispatch succeeds.
    last_err = None
    for attempt in range(3):
        try:
            res = run_bass_kernel_spmd(
                nc, in_maps, core_ids=list(range(N_CORES)), trace=trace
            )
            break
        except Exception as e:  # noqa: BLE001
            last_err = e
            import time as _time

            _time.sleep(2.0 * (attempt + 1))
    else:
        raise last_err

    rms = np.stack([res.results[k]["rm_d"] for k in range(N_CORES)])
    lmp = np.stack(
        [res.results[k]["lm"][0].astype(np.float64) for k in range(N_CORES)]
    )                                                       # [8, 26624]
    lmp = np.maximum.reduce(lmp, axis=0)                    # over cores
    neg = np.empty(N_PTS, np.float64)
    pg = lmp[:24576].reshape(3, 4, 2048)
    neg[:6144] = pg.max(axis=1).reshape(-1)                 # over pairs
    neg[6144:7680] = lmp[24576 : 24576 + 1536]
    neg[7680:] = lmp[26112 : 26112 + 512]

    pred_d2 = np.clip(-rms.astype(np.float64), 0.0, None)   # [8, 128, 8]
    pred_side = float(np.sqrt(pred_d2).sum()) / N_PTS
    lab_d2 = np.clip(-neg, 0.0, None)                       # [8192]
    lab_side = float(np.sqrt(lab_d2).sum()) / N_PTS
    return np.float32(pred_side + lab_side), res


def kernel(pred: np.ndarray, label: np.ndarray) -> np.ndarray:
    return _run(pred, label)[0]
